# revision 23
# baseline (speedup 1.0000x reference)
"""Block-FFT circulant matmul (BlockFFTDirectPrior) as a Trainium2 Bass kernel.

Math: out = ifft( einsum('bjf,ijf->bif', fft(x_blocks), conj(W_full)) ).real
with 64x64 blocks of size 256, batch 2048.

All matmul based:
  S1: DFT per input block j  (R stationary, x moving)       -> out1 [s,(b,hj)]
  P1: PE transpose s<->hj per batch row (is_transpose bf16) -> X2  [hj,(b,s)]
  S2: per spectrum slot s, mix blocks j->i (G_s stationary) -> O2  [m,(s,b)]
  P2: PE transpose m<->s per batch row (is_transpose bf16)  -> T2  [s,(b,m)]
  S3: IDFT (Ri stationary, data moving)                     -> Y2 [t',(b,i)]

Spectrum packing (256 real values per block): plane A = Re f=0..127,
plane B = [Re f=128, Im f=1..127]; slot s pairs (A[s], B[s]).

is_transpose matmuls emit bf16 directly into PSUM, so the permute
drains run in the DVE/ACT 2x packed mode.  Stage 3 keeps the inverse
DFT matrix stationary (4 weight loads per pass instead of 128) and
emits output transposed as [t', (b, i)]; the host undoes that
permutation for free.  A dummy matmul stream at kernel start warms the
PE clock (HAM) while the first inputs stream in.

Sharding: data-parallel over batch across 8 NeuronCores (256 rows each),
2 passes of 128 rows per core.
"""

import numpy as np
import ml_dtypes

import concourse.bass as bass
import concourse.mybir as mybir
from concourse import bacc
from concourse.tile import TileContext
from concourse.bass_utils import run_bass_kernel_spmd

B, KIN, KOUT, BLOCK = 2048, 64, 64, 256
NCORES = 8
BC = B // NCORES            # 256 batch rows per core
NPASS = 2
PB = BC // NPASS            # 128 batch rows per pass

F32 = mybir.dt.float32
BF16 = mybir.dt.bfloat16
NPBF16 = ml_dtypes.bfloat16

_NC_CACHE = {}


def _build_consts():
    """DFT / inverse-DFT matrices + identity, bf16, kernel layouts."""
    t = np.arange(BLOCK)
    f = np.arange(128)
    ang = 2.0 * np.pi * np.outer(t, f) / BLOCK          # [t, f]
    RA = np.cos(ang)                                    # re f=0..127
    RB = -np.sin(ang)                                   # im f=1..127
    RB[:, 0] = np.cos(np.pi * t)                        # re f=128 in col 0
    R = np.zeros((2, 2, 128, 128), dtype=NPBF16)        # [h, kt, t(128), s]
    for kt in range(2):
        R[0, kt] = RA[kt * 128:(kt + 1) * 128, :].astype(NPBF16)
        R[1, kt] = RB[kt * 128:(kt + 1) * 128, :].astype(NPBF16)

    s = np.arange(128)
    tp = np.arange(BLOCK)
    angi = 2.0 * np.pi * np.outer(s, tp) / BLOCK        # [s, t']
    w = np.full((128, 1), 2.0 / BLOCK)
    w[0] = 1.0 / BLOCK
    RiA = w * np.cos(angi)
    RiB = -(2.0 / BLOCK) * np.sin(angi)
    RiB[0, :] = (1.0 / BLOCK) * np.cos(np.pi * tp)      # Nyquist (real) term
    Ri = np.stack([RiA, RiB]).astype(NPBF16)            # [2, 128, 256]

    ident = np.eye(128, dtype=NPBF16)
    return R, Ri, ident


def _build_g(Wr, Wi):
    """Stage-2 mixing matrices, layout [k=(h*64+j), s, m=(re_i|im_i)], bf16."""
    G = np.zeros((128, 128, 128), dtype=np.float32)     # [s, k, m]
    G[0, :64, :64] = Wr[:, :, 0].T
    G[0, 64:, 64:] = Wr[:, :, 128].T
    WrT = np.transpose(Wr, (2, 1, 0))                   # [f, j, i]
    WiT = np.transpose(Wi, (2, 1, 0))
    G[1:, :64, :64] = WrT[1:128]
    G[1:, :64, 64:] = -WiT[1:128]
    G[1:, 64:, :64] = WiT[1:128]
    G[1:, 64:, 64:] = WrT[1:128]
    return np.ascontiguousarray(G.transpose(1, 0, 2)).astype(NPBF16)


def _build_nc():
    nc = bacc.Bacc("TRN2", target_bir_lowering=False, debug=False)
    # xP layout [pass, t(256), b(128), j(64)]
    xP = nc.dram_tensor("xP", [NPASS, BLOCK, PB, KIN], BF16, kind="ExternalInput")
    Gt = nc.dram_tensor("G", [128, 128 * 128], BF16, kind="ExternalInput")
    Rt = nc.dram_tensor("R", [2, 2, 128, 128], BF16, kind="ExternalInput")
    Rit = nc.dram_tensor("Ri", [2, 128, 256], BF16, kind="ExternalInput")
    It = nc.dram_tensor("Ident", [128, 128], BF16, kind="ExternalInput")
    # output, transposed: [pass, t'_half, t'_low(128), b(128)*i(64)]
    Y = nc.dram_tensor("Y", [NPASS, 2, 128, PB * KOUT], BF16,
                       kind="ExternalOutput")

    # greedy drain balancing between DVE (vector) and ACT (scalar).
    # bf16-psum drains go to DVE only (2x packed mode there; on ACT they
    # would thrash the activation table).  Costs are measured ns.
    bal = {"v": 0.0, "s": 0.0}

    def drain(dst, src, fd, accel=1):
        # bf16-psum 2x drains always on DVE (on ACT they thrash its
        # uop/table state); fp32 drains greedy-balanced with a lead cap
        # so neither engine starves within a phase
        if accel == 2:
            bal["v"] += 800.0 * fd / 1024
            nc.vector.tensor_copy(dst, src)
            return
        cv = 1400.0 * fd / 1024
        cs = 1150.0 * fd / 1024
        use_v = bal["v"] + cv <= bal["s"] + cs
        if bal["v"] - bal["s"] > 1200.0:
            use_v = False
        elif bal["s"] - bal["v"] > 1200.0:
            use_v = True
        if use_v:
            bal["v"] += cv
            nc.vector.tensor_copy(dst, src)
        else:
            bal["s"] += cs
            nc.scalar.copy(dst, src)

    with TileContext(nc) as tc:
        with (
            tc.tile_pool(name="const", bufs=1) as cpool,
            tc.tile_pool(name="big", bufs=1) as bigpool,
            tc.tile_pool(name="yt", bufs=2) as ypool,
            tc.tile_pool(name="psS", bufs=3, space="PSUM") as psS,
            tc.tile_pool(name="psP", bufs=2, space="PSUM") as psP,
        ):
            # ---------------- warmup + constants ----------------
            wz = cpool.tile([128, 512], BF16)
            nc.vector.memset(wz[:, :], 0.0)
            wps = psS.tile([128, 1024], F32, tag="s", name="warm")
            for _ in range(34):
                nc.tensor.matmul(wps[:, 0:512], wz[:, 0:128], wz[:, 0:512],
                                 start=True, stop=True)

            Rsb = cpool.tile([128, 4 * 128], BF16)
            for h in range(2):
                for kt in range(2):
                    nc.sync.dma_start(
                        Rsb[:, (h * 2 + kt) * 128:(h * 2 + kt + 1) * 128],
                        Rt.ap()[h, kt],
                    )
            Risb = cpool.tile([128, 512], BF16)
            for h in range(2):
                nc.sync.dma_start(Risb[:, h * 256:(h + 1) * 256], Rit.ap()[h])
            Isb = cpool.tile([128, 128], BF16)
            nc.sync.dma_start(Isb[:, :], It.ap())
            Gsb = cpool.tile([128, 128 * 128], BF16)

            # input loads, all on the sync HWDGE ring so they complete in
            # priority order: pass-0 x (kt-interleaved), then G, then pass-1 x
            xk = {}
            for p in range(NPASS):
                for kt in range(2):
                    xk[(p, kt)] = bigpool.tile(
                        [128, PB * KIN], BF16, tag=f"xk{kt}",
                        name=f"xk{kt}_{p}")
            for c in range(8):
                for kt in range(2):
                    nc.sync.dma_start(
                        xk[(0, kt)][:, c * 1024:(c + 1) * 1024],
                        xP.ap()[0, kt * 128:(kt + 1) * 128,
                                c * 16:(c + 1) * 16],
                    )
            for c in range(16):
                nc.sync.dma_start(Gsb[:, c * 1024:(c + 1) * 1024],
                                  Gt.ap()[:, c * 1024:(c + 1) * 1024])
            for c in range(8):
                for kt in range(2):
                    nc.sync.dma_start(
                        xk[(1, kt)][:, c * 1024:(c + 1) * 1024],
                        xP.ap()[1, kt * 128:(kt + 1) * 128,
                                c * 16:(c + 1) * 16],
                    )

            for p in range(NPASS):
                xk0, xk1 = xk[(p, 0)], xk[(p, 1)]
                # ---------------- S1: DFT per block ----------------
                # out1 [s, (b, hj)]: col = b*128 + h*64 + j
                out1 = bigpool.tile([128, 128 * PB], BF16, tag="out1",
                                    name=f"out1_{p}")
                out1v = out1.rearrange("p (b hj) -> p b hj", hj=128)
                for g2 in range(8):
                    for h in range(2):
                        ps = psS.tile([128, 1024], F32, tag="s",
                                      name=f"s1_{p}_{g2}_{h}")
                        for q in range(2):
                            g = g2 * 2 + q
                            nc.tensor.matmul(
                                ps[:, q * 512:(q + 1) * 512],
                                Rsb[:, (h * 2) * 128:(h * 2 + 1) * 128],
                                xk0[:, g * 512:(g + 1) * 512],
                                start=True, stop=False,
                            )
                        for q in range(2):
                            g = g2 * 2 + q
                            nc.tensor.matmul(
                                ps[:, q * 512:(q + 1) * 512],
                                Rsb[:, (h * 2 + 1) * 128:(h * 2 + 2) * 128],
                                xk1[:, g * 512:(g + 1) * 512],
                                start=False, stop=True,
                            )
                        # ps [s, (b16, j64)] -> out1 [s, b, h*64+j]
                        drain(
                            out1v[:, g2 * 16:(g2 + 1) * 16,
                                  h * 64:(h + 1) * 64],
                            ps.rearrange("p (b j) -> p b j", b=16),
                            1024,
                        )

                # ---- P1: out1 [s,(b,hj)] -> X2 [hj,(s,b)] ----
                # s-major X2 so the S2 moving operand is line-contiguous
                X2 = bigpool.tile([128, 128 * PB], BF16, tag="X2",
                                  name=f"X2_{p}")
                X2v = X2.rearrange("p (s b) -> p s b", b=PB)
                for b8 in range(16):
                    pt = psP.tile([128, 1024], BF16, tag="p",
                                  name=f"p1_{p}_{b8}")
                    for q in range(8):
                        b = b8 * 8 + q
                        nc.tensor.transpose(
                            pt[:, q * 128:(q + 1) * 128],
                            out1v[:, b, :], Isb[:, :],
                        )
                    # pt [hj, (b8, s)] -> X2 [hj, s, b8-range]
                    drain(X2v[:, :, b8 * 8:(b8 + 1) * 8],
                          pt.rearrange("p (b s) -> p s b", b=8),
                          1024)

                # ---------------- S2: mix blocks per slot ----------------
                # O2 [m, (b, s)]: b-major so the P2 stationary is contiguous
                O2 = bigpool.tile([128, 128 * PB], BF16, tag="O2",
                                  name=f"O2_{p}")
                O2v = O2.rearrange("p (b s) -> p b s", s=128)
                for sb in range(16):
                    ps = psS.tile([128, 1024], F32, tag="s",
                                  name=f"s2_{p}_{sb}")
                    for q in range(8):
                        s = sb * 8 + q
                        nc.tensor.matmul(
                            ps[:, q * 128:(q + 1) * 128],
                            Gsb[:, s * 128:(s + 1) * 128],
                            X2v[:, s, :],
                            start=True, stop=True,
                        )
                    # ps [m, (s8, b)] -> O2 [m, b, sb-range]
                    drain(O2v[:, :, sb * 8:(sb + 1) * 8],
                          ps.rearrange("p (s b) -> p b s", s=8),
                          1024)

                # ---- P2: O2 [m,(b,s)] -> T2 [s,(b,m)] ----
                T2 = bigpool.tile([128, 128 * PB], BF16, tag="T2",
                                  name=f"T2_{p}")
                for b8 in range(16):
                    pt = psP.tile([128, 1024], BF16, tag="p",
                                  name=f"p2_{p}_{b8}")
                    for q in range(8):
                        b = b8 * 8 + q
                        nc.tensor.transpose(
                            pt[:, q * 128:(q + 1) * 128],
                            O2v[:, b, :], Isb[:, :],
                        )
                    # pt [s, (b8, m)] -> T2 cols, contiguous
                    drain(T2[:, b8 * 1024:(b8 + 1) * 1024], pt[:, :],
                          1024, accel=2)

                # ---------- S3: IDFT, Ri stationary ---------------
                # T2 [s, (b, h, i)]; out [t', (b, i)] per t'-half
                T2v = T2.rearrange("p (b h i) -> p b h i", h=2, i=64)
                for th in range(2):
                    for c2 in range(4):
                        yt = ypool.tile([128, 2048], BF16, tag="yt",
                                        name=f"yt_{p}_{th}_{c2}")
                        for cc in range(2):
                            c = c2 * 2 + cc
                            ps = psS.tile([128, 1024], F32, tag="s",
                                          name=f"s3_{p}_{th}_{c}")
                            for h in range(2):
                                for q in range(2):
                                    b0 = c * 16 + q * 8
                                    nc.tensor.matmul(
                                        ps[:, q * 512:(q + 1) * 512],
                                        Risb[:, h * 256 + th * 128:
                                             h * 256 + (th + 1) * 128],
                                        T2v[:, b0:b0 + 8, h, :],
                                        start=(h == 0), stop=(h == 1),
                                    )
                            # ps [t', (b16, i64)]
                            drain(yt[:, cc * 1024:(cc + 1) * 1024],
                                  ps[:, :], 1024)
                        eng = nc.sync if (th * 4 + c2) % 2 == 0 else nc.scalar
                        eng.dma_start(
                            Y.ap()[p, th, :,
                                   c2 * 2048:(c2 + 1) * 2048],
                            yt[:, :],
                        )
    nc.compile()
    return nc


def _get_nc():
    if "nc" not in _NC_CACHE:
        _NC_CACHE["nc"] = _build_nc()
    return _NC_CACHE["nc"]


def run(x, W_real, W_imag, trace=False):
    x = np.asarray(x, dtype=np.float32)
    Wr = np.asarray(W_real, dtype=np.float32)
    Wi = np.asarray(W_imag, dtype=np.float32)

    nc = _get_nc()
    R, Ri, ident = _build_consts()
    G = _build_g(Wr, Wi).reshape(128, 128 * 128)

    in_maps = []
    for c in range(NCORES):
        xc = x[c * BC:(c + 1) * BC]                       # [256, 16384]
        # -> [t, b, j] -> [pass, t(256), b(128), j(64)]
        xcp = xc.reshape(BC, KIN, BLOCK).transpose(2, 0, 1)
        xcp = xcp.reshape(BLOCK, NPASS, PB, KIN).transpose(1, 0, 2, 3)
        in_maps.append({
            "xP": np.ascontiguousarray(xcp).astype(NPBF16),
            "G": G, "R": R, "Ri": Ri, "Ident": ident,
        })
    res = run_bass_kernel_spmd(
        nc, in_maps, core_ids=list(range(NCORES)), trace=trace
    )
    outs = []
    for r in res.results:
        y = np.asarray(r["Y"], dtype=np.float32)          # [p, th, tl, b*64]
        y = y.reshape(NPASS, 2, 128, PB, KOUT)            # [p, th, tl, b, i]
        y = y.transpose(0, 3, 4, 1, 2)                    # [p, b, i, th, tl]
        outs.append(np.ascontiguousarray(y).reshape(BC, KOUT * BLOCK))
    out = np.concatenate(outs, axis=0)
    return np.ascontiguousarray(out, dtype=np.float32), res


def kernel(x, W_real, W_imag):
    out, _ = run(x, W_real, W_imag)
    return out
